# revision 21
# baseline (speedup 1.0000x reference)
"""Trainium2 Bass kernel for nn_CachedCompressedLinear.

out[16, 11008] = x[16, 4096] @ ((w_q - 128) * scale).T + bias

w_q is int32 but carries only 8 bits (codes 0..255).  The host packs
codes to 1 byte, cutting HBM traffic ~4x.  The PE is the true floor
(44032 moving bf16 columns ~ 18.6 us warm), so the design keeps the
PE warm and never starved:
  - 9 dummy warm-up matmuls fill the ~7 us framework preamble and
    bridge to the first decoded tile, so the HAM clock is at 2.4 GHz
    for the whole real stream (no mid-run re-throttle);
  - the weight stream uses granular DMAs early (1/1/2/2/2 k-tiles,
    then 4s, 2s at the tail) so DMA-completion latency stays under
    the PE per-tile deadline;
  - decode is 2-tile ops: DVE (~797 ns/tile) carries the front, ACT
    (~1286 ns/tile, ~2.6 us/op latency) joins from k=12;
  - small tail DMAs + a staggered per-chunk close keep the end chain
    short.
scale is folded into x on the host (bf16; rel-err ~2^-10 vs the 2e-2
gate) so the epilogue is a bare PSUM->SBUF copy; bias enters PSUM via
a K=2 one-hot matmul of a host-prepacked bf16 hi/lo pair.  Every DMA
and decode op owns its SBUF slot (no WAR stalls).

Sharding: column-parallel over 8 cores, 1376 out-features each.
"""

import sys

if "/opt/trn_rl_repo" not in sys.path:
    sys.path.insert(0, "/opt/trn_rl_repo")

import numpy as np
import ml_dtypes

IN_F = 4096
OUT_F = 11008
BATCH = 16
N_CORES = 8
O_PER = 1376
K_TILES = 32
M = 16
CHUNKS = [(0, 512), (512, 512), (1024, 352)]
NBF = 2  # k0,k1 travel as ready-to-use bf16 (saves 2 DVE ops + decode latency)

# int8 weight DMAs (k0, ntiles); granular early so decode latency stays
# under the PE per-tile deadline, small at the tail for a short close.
DMAS8 = [(2, 2), (4, 4), (8, 4), (12, 4), (16, 4),
         (20, 4), (24, 4), (28, 2), (30, 2)]
# decode ops (k0, ntiles, engine); ACT's ~2.6us op latency only fits
# from k12 on, DVE (876/1593 ns ops) carries the front.
OPS = [
    (2, 2, "dve"), (4, 2, "dve"),
    (6, 2, "dve"), (8, 2, "dve"), (10, 2, "dve"),
    (12, 2, "act"), (14, 2, "dve"), (16, 2, "act"), (18, 2, "dve"),
    (20, 2, "act"), (22, 2, "dve"), (24, 2, "act"), (26, 2, "dve"),
    (28, 2, "dve"), (30, 2, "dve"),
]
NWARM = 9

_BUILT = None


def _build():
    import concourse.bass as bass
    import concourse.tile as tile
    from concourse import bacc, mybir

    dt = mybir.dt
    nc = bacc.Bacc("TRN2", target_bir_lowering=False, debug=False)

    wb16 = nc.dram_tensor(
        "wb16", [128, NBF, O_PER], dt.bfloat16, kind="ExternalInput"
    )
    w8 = nc.dram_tensor(
        "w8", [128, K_TILES - NBF, O_PER], dt.int8, kind="ExternalInput"
    )
    xpk = nc.dram_tensor(
        "xpk", [128, (K_TILES + 1) * M], dt.bfloat16, kind="ExternalInput"
    )
    bias_hl = nc.dram_tensor("bias_hl", [2, O_PER], dt.bfloat16, kind="ExternalInput")
    out = nc.dram_tensor("out", [BATCH, O_PER], dt.float32, kind="ExternalOutput")

    BIASBLK = K_TILES

    with tile.TileContext(nc) as tc:
        with (
            tc.tile_pool(name="consts", bufs=1) as consts,
            tc.tile_pool(name="w8p", bufs=len(DMAS8)) as w8p,
            tc.tile_pool(name="wbf", bufs=len(OPS)) as wbfp,
            tc.tile_pool(name="psum", bufs=1, space=bass.MemorySpace.PSUM) as psump,
            tc.tile_pool(name="outp", bufs=1) as outp,
        ):
            # x/bias first (small, needed by the first matmul), then the
            # bf16 opener pair (k0,k1: no decode latency), then int8 weights
            x_sb = consts.tile([128, (K_TILES + 1) * M], dt.bfloat16)
            nc.sync.dma_start(x_sb[:], xpk[:])
            bias_sb = consts.tile([2, O_PER], dt.bfloat16)
            nc.sync.dma_start(bias_sb[:], bias_hl[:])
            bf01 = consts.tile([128, NBF, O_PER], dt.bfloat16, name="bf01")
            nc.sync.dma_start(bf01[:], wb16[:])

            w8_ts = {}
            for k0, n in DMAS8:
                w8_ts[k0] = w8p.tile(
                    [128, 4, O_PER], dt.int8, name=f"w8_{k0}", tag="w8"
                )
                nc.sync.dma_start(
                    w8_ts[k0][:, 0:n, :], w8[:][:, k0 - NBF : k0 - NBF + n, :]
                )

            def dma_tile_for(k):
                for d0, dn in DMAS8:
                    if d0 <= k < d0 + dn:
                        return w8_ts[d0], k - d0
                raise AssertionError(k)

            # PE warm-up on junk data
            junkw = consts.tile([128, M], dt.bfloat16, name="junkw")
            nc.vector.memset(junkw[:], 0.0)
            junkm = consts.tile([128, 512], dt.bfloat16, name="junkm")
            nc.vector.memset(junkm[:], 0.0)

            # one PSUM bank per chunk (bank-overlap tracker would serialize
            # the close if they shared one), chunk i at partitions 32i..32i+16
            pss = [
                psump.tile([128, 512], dt.float32, name=f"ps{i}", tag=f"ps{i}")
                for i in range(3)
            ]
            ps_dummy = psump.tile([M, 512], dt.float32, name="psd", tag="psd")
            for _ in range(NWARM):
                nc.tensor.matmul(
                    ps_dummy[:, :], junkw[:], junkm[:], start=True, stop=True
                )

            def mm_tile(k, wsrc, start=False):
                # the 3 o-chunks run CONCURRENTLY in distinct 32-column
                # strips of the PE array (col tiling, M=16 << 32)
                for i, (o, w) in enumerate(CHUNKS):
                    nc.tensor.matmul(
                        pss[i][32 * i : 32 * i + M, 0:w],
                        x_sb[:, k * M : (k + 1) * M],
                        wsrc[:, o : o + w],
                        start=start,
                        stop=False,
                        tile_position=(0, 32 * i),
                        skip_group_check=True,
                    )
                if start:
                    for i, (o, w) in enumerate(CHUNKS):
                        nc.tensor.matmul(
                            pss[i][32 * i : 32 * i + M, 0:w],
                            x_sb[0:2, BIASBLK * M : BIASBLK * M + M],
                            bias_sb[0:2, o : o + w],
                            start=False,
                            stop=False,
                            tile_position=(0, 32 * i),
                            skip_group_check=True,
                        )

            # bf16 openers: straight to the PE
            mm_tile(0, bf01[:, 0, :], start=True)
            mm_tile(1, bf01[:, 1, :])

            # int8 stream: decode then matmul
            first = False
            wb_last = None
            for k0, n, eng in OPS:
                src_t, off = dma_tile_for(k0)
                wb_t = wbfp.tile(
                    [128, 2, O_PER], dt.bfloat16, name=f"wb_{k0}", tag="wb"
                )
                if eng == "dve":
                    nc.vector.tensor_copy(
                        wb_t[:, 0:n, :], src_t[:, off : off + n, :]
                    )
                else:
                    nc.scalar.copy(wb_t[:, 0:n, :], src_t[:, off : off + n, :])
                for j in range(n):
                    k = k0 + j
                    if k == 31:
                        wb_last = wb_t[:, j, :]
                        continue  # staggered close below
                    mm_tile(k, wb_t[:, j, :], start=first)
                    first = False

            # staggered close: per chunk, stop matmul (k=31) -> copy -> DMA
            for i, (o, w) in enumerate(CHUNKS):
                nc.tensor.matmul(
                    pss[i][32 * i : 32 * i + M, 0:w],
                    x_sb[:, 31 * M : 32 * M],
                    wb_last[:, o : o + w],
                    start=False,
                    stop=True,
                    tile_position=(0, 32 * i),
                    skip_group_check=True,
                )
                ob = outp.tile([BATCH, w], dt.float32, name=f"ob{i}")
                if i == 1:
                    nc.vector.tensor_copy(ob[:], pss[i][32 * i : 32 * i + M, 0:w])
                else:
                    nc.scalar.copy(ob[:], pss[i][32 * i : 32 * i + M, 0:w])
                nc.sync.dma_start(out[:][:, o : o + w], ob[:])

    nc.compile()
    return nc


def _get_built():
    global _BUILT
    if _BUILT is None:
        _BUILT = _build()
    return _BUILT


def make_in_maps(x, w_q, scale, bias):
    """Host-side shard + layout prep. Returns per-core input dicts."""
    x = np.asarray(x, dtype=np.float32)
    w_q = np.asarray(w_q, dtype=np.int32)
    scale = np.asarray(scale, dtype=np.float32)
    bias = np.asarray(bias, dtype=np.float32)
    s = float(scale.reshape(-1)[0])

    xsT = np.ascontiguousarray((x * s).T)  # [4096, 16] f32
    xpk = np.zeros((128, (K_TILES + 1) * M), dtype=ml_dtypes.bfloat16)
    xpk[:, : K_TILES * M] = (
        xsT.reshape(K_TILES, 128, M)
        .transpose(1, 0, 2)
        .reshape(128, K_TILES * M)
        .astype(ml_dtypes.bfloat16)
    )
    xpk[0:2, K_TILES * M : K_TILES * M + BATCH] = 1.0  # bias one-hot rows

    w8all = (w_q - 128).astype(np.int8)

    bh32 = bias.astype(ml_dtypes.bfloat16).astype(np.float32)
    bl = (bias - bh32).astype(ml_dtypes.bfloat16)

    in_maps = []
    for c in range(N_CORES):
        wt = (
            w8all[c * O_PER : (c + 1) * O_PER]
            .T.reshape(K_TILES, 128, O_PER)
            .transpose(1, 0, 2)
        )  # [128, 32, 1376] int8
        wb16_c = np.ascontiguousarray(wt[:, :NBF, :].astype(ml_dtypes.bfloat16))
        w8_c = np.ascontiguousarray(wt[:, NBF:, :])
        bias_hl_c = np.empty((2, O_PER), dtype=ml_dtypes.bfloat16)
        bias_hl_c[0] = bh32[c * O_PER : (c + 1) * O_PER].astype(ml_dtypes.bfloat16)
        bias_hl_c[1] = bl[c * O_PER : (c + 1) * O_PER]
        in_maps.append(
            {"wb16": wb16_c, "w8": w8_c, "xpk": xpk, "bias_hl": bias_hl_c}
        )
    return in_maps


def run(inputs, trace=False):
    """Run on the 8 NeuronCores. Returns (full_output, BassKernelResults)."""
    from concourse.bass_utils import run_bass_kernel_spmd

    in_maps = make_in_maps(**inputs)
    nc = _get_built()
    res = run_bass_kernel_spmd(nc, in_maps, list(range(N_CORES)), trace=trace)
    parts = [np.asarray(res.results[c]["out"]) for c in range(N_CORES)]
    full = np.concatenate(parts, axis=1).astype(np.float32)
    return full, res


def kernel(**inputs) -> np.ndarray:
    full, _ = run(inputs, trace=False)
    return full


# revision 23
# speedup vs baseline: 1.0205x; 1.0205x over previous
"""Trainium2 Bass kernel for nn_CachedCompressedLinear.

out[16, 11008] = x[16, 4096] @ ((w_q - 128) * scale).T + bias

w_q is int32 but carries only 8 bits (codes 0..255).  The host packs
codes to 1 byte, cutting HBM traffic ~4x.  The PE is the true floor
(44032 moving bf16 columns ~ 18.6 us warm), so the design keeps the
PE warm and never starved:
  - 9 dummy warm-up matmuls fill the ~7 us framework preamble and
    bridge to the first decoded tile, so the HAM clock is at 2.4 GHz
    for the whole real stream (no mid-run re-throttle);
  - the weight stream uses granular DMAs early (1/1/2/2/2 k-tiles,
    then 4s, 2s at the tail) so DMA-completion latency stays under
    the PE per-tile deadline;
  - decode is 2-tile ops: DVE (~797 ns/tile) carries the front, ACT
    (~1286 ns/tile, ~2.6 us/op latency) joins from k=12;
  - small tail DMAs + a staggered per-chunk close keep the end chain
    short.
scale is folded into x on the host (bf16; rel-err ~2^-10 vs the 2e-2
gate) so the epilogue is a bare PSUM->SBUF copy; bias enters PSUM via
a K=2 one-hot matmul of a host-prepacked bf16 hi/lo pair.  Every DMA
and decode op owns its SBUF slot (no WAR stalls).

Sharding: column-parallel over 8 cores, 1376 out-features each.
"""

import sys

if "/opt/trn_rl_repo" not in sys.path:
    sys.path.insert(0, "/opt/trn_rl_repo")

import numpy as np
import ml_dtypes

IN_F = 4096
OUT_F = 11008
BATCH = 16
N_CORES = 8
O_PER = 1376
K_TILES = 32
M = 16
CHUNKS = [(0, 512), (512, 512), (1024, 352)]
NBF = 2  # k0,k1 travel as ready-to-use bf16 (saves 2 DVE ops + decode latency)

# int8 weight DMAs (k0, ntiles); granular early so decode latency stays
# under the PE per-tile deadline, small at the tail for a short close.
DMAS8 = [(2, 2), (4, 2), (6, 2), (8, 4), (12, 4), (16, 4),
         (20, 4), (24, 4), (28, 2), (30, 2)]
# decode ops (k0, ntiles, engine); ACT's ~2.6us op latency only fits
# from k12 on, DVE (876/1593 ns ops) carries the front.
OPS = [
    (2, 2, "dve"), (4, 2, "dve"),
    (6, 2, "dve"), (8, 2, "dve"), (10, 2, "dve"),
    (12, 2, "act"), (14, 2, "dve"), (16, 2, "act"), (18, 2, "dve"),
    (20, 2, "act"), (22, 2, "dve"), (24, 2, "act"), (26, 2, "dve"),
    (28, 2, "dve"), (30, 1, "dve"), (31, 1, "dve"),
]
NWARM = 9

_BUILT = None


def _build():
    import concourse.bass as bass
    import concourse.tile as tile
    from concourse import bacc, mybir

    dt = mybir.dt
    nc = bacc.Bacc("TRN2", target_bir_lowering=False, debug=False)

    wb16 = nc.dram_tensor(
        "wb16", [128, NBF, O_PER], dt.bfloat16, kind="ExternalInput"
    )
    w8 = nc.dram_tensor(
        "w8", [128, K_TILES - NBF, O_PER], dt.int8, kind="ExternalInput"
    )
    xpk = nc.dram_tensor(
        "xpk", [128, (K_TILES + 1) * M], dt.bfloat16, kind="ExternalInput"
    )
    bias_hl = nc.dram_tensor("bias_hl", [2, O_PER], dt.bfloat16, kind="ExternalInput")
    out = nc.dram_tensor("out", [BATCH, O_PER], dt.float32, kind="ExternalOutput")

    BIASBLK = K_TILES

    with tile.TileContext(nc) as tc:
        with (
            tc.tile_pool(name="consts", bufs=1) as consts,
            tc.tile_pool(name="w8p", bufs=len(DMAS8)) as w8p,
            tc.tile_pool(name="wbf", bufs=len(OPS)) as wbfp,
            tc.tile_pool(name="psum", bufs=1, space=bass.MemorySpace.PSUM) as psump,
            tc.tile_pool(name="outp", bufs=1) as outp,
        ):
            # x/bias first (small), then the first two int8 DMAs (they gate
            # the saturated DVE decode chain - the critical path), THEN the
            # bf16 opener pair (its PE consumers have slack), then the rest.
            x_sb = consts.tile([128, (K_TILES + 1) * M], dt.bfloat16)
            nc.sync.dma_start(x_sb[:], xpk[:])
            bias_sb = consts.tile([2, O_PER], dt.bfloat16)
            nc.sync.dma_start(bias_sb[:], bias_hl[:])

            w8_ts = {}

            def w8_dma(k0, n):
                w8_ts[k0] = w8p.tile(
                    [128, 4, O_PER], dt.int8, name=f"w8_{k0}", tag="w8"
                )
                nc.sync.dma_start(
                    w8_ts[k0][:, 0:n, :], w8[:][:, k0 - NBF : k0 - NBF + n, :]
                )

            for k0, n in DMAS8[:2]:
                w8_dma(k0, n)
            bf01 = consts.tile([128, NBF, O_PER], dt.bfloat16, name="bf01")
            nc.sync.dma_start(bf01[:], wb16[:])
            for k0, n in DMAS8[2:]:
                w8_dma(k0, n)

            def dma_tile_for(k):
                for d0, dn in DMAS8:
                    if d0 <= k < d0 + dn:
                        return w8_ts[d0], k - d0
                raise AssertionError(k)

            # PE warm-up on junk data
            junkw = consts.tile([128, M], dt.bfloat16, name="junkw")
            nc.vector.memset(junkw[:], 0.0)
            junkm = consts.tile([128, 512], dt.bfloat16, name="junkm")
            nc.vector.memset(junkm[:], 0.0)

            # one PSUM bank per chunk (bank-overlap tracker would serialize
            # the close if they shared one), chunk i at partitions 32i..32i+16
            pss = [
                psump.tile([128, 512], dt.float32, name=f"ps{i}", tag=f"ps{i}")
                for i in range(3)
            ]
            ps_dummy = psump.tile([M, 512], dt.float32, name="psd", tag="psd")
            for _ in range(NWARM):
                nc.tensor.matmul(
                    ps_dummy[:, :], junkw[:], junkm[:], start=True, stop=True
                )

            def mm_tile(k, wsrc, start=False):
                # the 3 o-chunks run CONCURRENTLY in distinct 32-column
                # strips of the PE array (col tiling, M=16 << 32)
                for i, (o, w) in enumerate(CHUNKS):
                    nc.tensor.matmul(
                        pss[i][32 * i : 32 * i + M, 0:w],
                        x_sb[:, k * M : (k + 1) * M],
                        wsrc[:, o : o + w],
                        start=start,
                        stop=False,
                        tile_position=(0, 32 * i),
                        skip_group_check=True,
                    )
                if start:
                    for i, (o, w) in enumerate(CHUNKS):
                        nc.tensor.matmul(
                            pss[i][32 * i : 32 * i + M, 0:w],
                            x_sb[0:2, BIASBLK * M : BIASBLK * M + M],
                            bias_sb[0:2, o : o + w],
                            start=False,
                            stop=False,
                            tile_position=(0, 32 * i),
                            skip_group_check=True,
                        )

            # bf16 openers: straight to the PE
            mm_tile(0, bf01[:, 0, :], start=True)
            mm_tile(1, bf01[:, 1, :])

            # int8 stream: decode then matmul
            first = False
            wb_last = None
            for k0, n, eng in OPS:
                src_t, off = dma_tile_for(k0)
                wb_t = wbfp.tile(
                    [128, 2, O_PER], dt.bfloat16, name=f"wb_{k0}", tag="wb"
                )
                if eng == "dve":
                    nc.vector.tensor_copy(
                        wb_t[:, 0:n, :], src_t[:, off : off + n, :]
                    )
                else:
                    nc.scalar.copy(wb_t[:, 0:n, :], src_t[:, off : off + n, :])
                for j in range(n):
                    k = k0 + j
                    if k == 31:
                        wb_last = wb_t[:, j, :]
                        continue  # staggered close below
                    mm_tile(k, wb_t[:, j, :], start=first)
                    first = False

            # staggered close: per chunk, stop matmul (k=31) -> copy -> DMA
            for i, (o, w) in enumerate(CHUNKS):
                nc.tensor.matmul(
                    pss[i][32 * i : 32 * i + M, 0:w],
                    x_sb[:, 31 * M : 32 * M],
                    wb_last[:, o : o + w],
                    start=False,
                    stop=True,
                    tile_position=(0, 32 * i),
                    skip_group_check=True,
                )
                ob = outp.tile([BATCH, w], dt.float32, name=f"ob{i}")
                if i == 1:
                    nc.vector.tensor_copy(ob[:], pss[i][32 * i : 32 * i + M, 0:w])
                else:
                    nc.scalar.copy(ob[:], pss[i][32 * i : 32 * i + M, 0:w])
                nc.sync.dma_start(out[:][:, o : o + w], ob[:])

    nc.compile()
    return nc


def _get_built():
    global _BUILT
    if _BUILT is None:
        _BUILT = _build()
    return _BUILT


def make_in_maps(x, w_q, scale, bias):
    """Host-side shard + layout prep. Returns per-core input dicts."""
    x = np.asarray(x, dtype=np.float32)
    w_q = np.asarray(w_q, dtype=np.int32)
    scale = np.asarray(scale, dtype=np.float32)
    bias = np.asarray(bias, dtype=np.float32)
    s = float(scale.reshape(-1)[0])

    xsT = np.ascontiguousarray((x * s).T)  # [4096, 16] f32
    xpk = np.zeros((128, (K_TILES + 1) * M), dtype=ml_dtypes.bfloat16)
    xpk[:, : K_TILES * M] = (
        xsT.reshape(K_TILES, 128, M)
        .transpose(1, 0, 2)
        .reshape(128, K_TILES * M)
        .astype(ml_dtypes.bfloat16)
    )
    xpk[0:2, K_TILES * M : K_TILES * M + BATCH] = 1.0  # bias one-hot rows

    w8all = (w_q - 128).astype(np.int8)

    bh32 = bias.astype(ml_dtypes.bfloat16).astype(np.float32)
    bl = (bias - bh32).astype(ml_dtypes.bfloat16)

    in_maps = []
    for c in range(N_CORES):
        wt = (
            w8all[c * O_PER : (c + 1) * O_PER]
            .T.reshape(K_TILES, 128, O_PER)
            .transpose(1, 0, 2)
        )  # [128, 32, 1376] int8
        wb16_c = np.ascontiguousarray(wt[:, :NBF, :].astype(ml_dtypes.bfloat16))
        w8_c = np.ascontiguousarray(wt[:, NBF:, :])
        bias_hl_c = np.empty((2, O_PER), dtype=ml_dtypes.bfloat16)
        bias_hl_c[0] = bh32[c * O_PER : (c + 1) * O_PER].astype(ml_dtypes.bfloat16)
        bias_hl_c[1] = bl[c * O_PER : (c + 1) * O_PER]
        in_maps.append(
            {"wb16": wb16_c, "w8": w8_c, "xpk": xpk, "bias_hl": bias_hl_c}
        )
    return in_maps


def run(inputs, trace=False):
    """Run on the 8 NeuronCores. Returns (full_output, BassKernelResults)."""
    from concourse.bass_utils import run_bass_kernel_spmd

    in_maps = make_in_maps(**inputs)
    nc = _get_built()
    res = run_bass_kernel_spmd(nc, in_maps, list(range(N_CORES)), trace=trace)
    parts = [np.asarray(res.results[c]["out"]) for c in range(N_CORES)]
    full = np.concatenate(parts, axis=1).astype(np.float32)
    return full, res


def kernel(**inputs) -> np.ndarray:
    full, _ = run(inputs, trace=False)
    return full
